# revision 32
# baseline (speedup 1.0000x reference)
"""Masked self-attention (softmax over axis=1) Bass kernel for TRN2, 8 cores.

Reference semantics (per batch b):
    attn[l, m] = <a_l, a_m> * temperature            [L, L]
    attn = where(mask[l, m], attn, -1e7)
    P = softmax(attn, axis=l)                        (softmax over dim 0)
    out[m, :] = sum_l P[l, m] * a[l, :]              [L, H]

v4 design (pure data parallel, 4 batches/core, no collectives):

  Software-pipelined emission: staging for batch b+1 is emitted before
  compute of batch b, so the scheduler and the DMA ring FIFOs pull the
  staging chain a full batch ahead of where it's consumed.

  Staging per batch (chunked):
    asb f32 <- scalar-ring loads (batch 0: alternating sync/scalar)
    t2 bf16 [a|1] <- DVE/ACT casts per chunk
    scratch DRAM bf16 <- bounce t2 out on gpsimd SWDGE (batch 0:
        alternating scalar/gpsimd)
    AT bf16 <- xbar transpose DMAs (sync ring: transposes only after
        batch-0's early loads - mode switches serialize a ring)
    AT8 fp8e4 <- DVE casts per xbar chunk
    mask <- gpsimd SWDGE
  S = AT8^T @ AT8 in fp8 DoubleRow (2 d-tiles/pass, 3 passes)
  S' = mask_u8 * (BIG/temp) + S    (DVE scalar_tensor_tensor)
  E = exp(temp*S' - BIG) -> bf16   (ACT)
  [feat | denom] = E^T @ [t2 | 1]  (PE bf16, paired-chunk weight reuse)
  out = feat * (1/denom): DVE reciprocal + scale-copy alternating
      ACT / DVE per m-tile; stores alternate gpsimd/scalar rings.
  PE warmup: dummy matmuls during batch-0 staging flip the HAM clock
      gate to 8/8 before real work lands.
"""

import os as _os
import sys

import numpy as np

sys.path.insert(0, "/opt/trn_rl_repo")

B, L, H = 32, 1024, 768
N_CORES = 8
B_LOCAL = B // N_CORES  # 4 batches per core
LT = L // 128  # 8 l-tiles
DT = H // 128  # 6 d-tiles
DP = DT // 2  # 3 d-tile pairs (DoubleRow)
BIG = 50.0

N_TRANSPOSE = int(_os.environ.get("K_NT", "2"))  # xbar chunks per batch
N_STAGE = int(_os.environ.get("K_NST", "4"))  # a-load chunks
WARM = int(_os.environ.get("K_WARM", "1"))  # PE warmup matmuls
NORM_DVE = int(_os.environ.get("K_NDVE", "0"))  # keep normalize on ACT; DVE carries the transpose copies
ST_DELAY = float(_os.environ.get("K_STD", "0.010"))  # ms, staging-1 hold
MASK_DELAY = float(_os.environ.get("K_MKD", "0.014"))  # ms, mask-0 hold

_CACHE = {}


def _build(temp: float, repeats: int = 1, bench: bool = False):
    from contextlib import ExitStack

    import concourse.mybir as mybir
    from concourse import bacc, tile

    f32 = mybir.dt.float32
    bf16 = mybir.dt.bfloat16
    fp8 = mybir.dt.float8e4
    u8 = mybir.dt.uint8
    DR = mybir.MatmulPerfMode.DoubleRow

    nc = bacc.Bacc(
        "TRN2", target_bir_lowering=False, debug=False, num_devices=N_CORES
    )

    if bench:
        # Timing-only variant: big tensors live in Internal DRAM (content
        # irrelevant - instruction stream is identical), so per-call axon
        # transfer overhead stays tiny and the R-repeat delta is clean.
        nc.dram_tensor("bench_in", [1, 4], f32, kind="ExternalInput")
        nc.dram_tensor("out", [1, 4], f32, kind="ExternalOutput")
        a_ext = nc.dram_tensor("a", [B_LOCAL, L, H], f32).ap()
        m_ext = nc.dram_tensor("mask_a", [B_LOCAL, L, L], u8).ap()
        out_ext = nc.dram_tensor("out_int", [B_LOCAL, L, H], f32).ap()
    else:
        a_ext = nc.dram_tensor("a", [B_LOCAL, L, H], f32, kind="ExternalInput").ap()
        m_ext = nc.dram_tensor(
            "mask_a", [B_LOCAL, L, L], u8, kind="ExternalInput"
        ).ap()
        out_ext = nc.dram_tensor(
            "out", [B_LOCAL, L, H], f32, kind="ExternalOutput"
        ).ap()

    big_over_temp = BIG / temp

    with tile.TileContext(nc) as tc, ExitStack() as ctx:
        a_pool = ctx.enter_context(tc.tile_pool(name="asb", bufs=2))
        t2_pool = ctx.enter_context(tc.tile_pool(name="t2", bufs=3))
        at_pool = ctx.enter_context(tc.tile_pool(name="at", bufs=2))
        at8_pool = ctx.enter_context(tc.tile_pool(name="at8", bufs=2))
        mask_pool = ctx.enter_context(tc.tile_pool(name="mask", bufs=2))
        e_pool = ctx.enter_context(tc.tile_pool(name="e", bufs=2))
        sp_pool = ctx.enter_context(tc.tile_pool(name="sp", bufs=4))
        out_pool = ctx.enter_context(tc.tile_pool(name="outp", bufs=3))
        rc_pool = ctx.enter_context(tc.tile_pool(name="rc", bufs=3))
        dram_pool = ctx.enter_context(
            tc.tile_pool(name="bounce", bufs=2, space="DRAM")
        )
        psum_s = ctx.enter_context(tc.tile_pool(name="ps_s", bufs=2, space="PSUM"))
        psum_o = ctx.enter_context(tc.tile_pool(name="ps_o", bufs=2, space="PSUM"))
        const_pool = ctx.enter_context(tc.tile_pool(name="const", bufs=1))

        neg_big = const_pool.tile([128, 1], f32)
        nc.vector.memset(neg_big[:], -BIG)
        if WARM:
            wz = const_pool.tile([128, 512], bf16)
            nc.vector.memset(wz[:], 0.0)
        from concourse.masks import make_identity

        ident = const_pool.tile([128, 128], bf16)
        make_identity(nc, ident[:])

        def emit_staging(bi, b):
            a_v = a_ext[b].rearrange("(i p) d -> p i d", p=128)  # [128, 8, 768]
            m_v = m_ext[b].rearrange("(i p) m -> p i m", p=128)

            asb = a_pool.tile([128, LT, H], f32)
            t2 = t2_pool.tile([128, LT, H + 1], bf16)
            at8 = at8_pool.tile([128, DT, L], fp8)
            msk = mask_pool.tile([128, LT, L], u8)

            # a loads + t2 casts; batch 0 splits loads across both HWDGE
            # rings to halve the fill.
            nst = 8 if bi == 0 else N_STAGE
            lchunk = LT // nst
            for ci in range(nst):
                sl = slice(lchunk * ci, lchunk * (ci + 1))
                ld_eng = nc.sync if (bi == 0 and ci % 2 == 0) else nc.scalar
                ld_eng.dma_start(out=asb[:, sl, :], in_=a_v[:, sl, :])
                if bi == 0 or ci % 2 == 0:
                    # batch 0: all casts on DVE (faster than ACT Copy) in
                    # fine chunks - the l-major transposes chase them
                    nc.vector.tensor_copy(t2[:, sl, 0:H], asb[:, sl, :])
                else:
                    nc.scalar.activation(
                        out=t2[:, sl, 0:H],
                        in_=asb[:, sl, :],
                        func=mybir.ActivationFunctionType.Copy,
                    )
            nc.vector.memset(t2[:, :, H : H + 1], 1.0)
            # PE block-transposes of t2 build AT8 for every batch: no
            # DRAM bounce, no xbar, no aliased DMA-semaphore waits. ~4.3us
            # of PE per batch, emitted ahead of the batch's S so the tile
            # scheduler streams them behind the previous batch's MM2.
            for dj in range(DT):
                tpf = psum_o.tile([128, H + 1], f32, tag="po")
                tp = tpf[:].bitcast(bf16)
                for li in range(LT):
                    nc.tensor.transpose(
                        tp[:, 128 * li : 128 * (li + 1)],
                        t2[:, li, 128 * dj : 128 * (dj + 1)],
                        ident[:],
                    )
                # one contiguous [128, 1024] PSUM->SBUF copy per d-tile
                nc.vector.tensor_copy(at8[:, dj, :], tp[:, 0:L])
            # mask (gpsimd SWDGE)
            with tc.tile_wait_until(MASK_DELAY, enable=bi == 0 and MASK_DELAY > 0):
                for ci in range(2):
                    sl = slice(4 * ci, 4 * (ci + 1))
                    nc.gpsimd.dma_start(out=msk[:, sl, :], in_=m_v[:, sl, :])
            return dict(t2=t2, at8=at8, msk=msk, bi=bi)

        def emit_compute(bi, b, st, last=False):
            t2, at8, msk = st["t2"], st["at8"], st["msk"]
            o_v = out_ext[b].rearrange("(i p) d -> p i d", p=128)
            # S rows: fp8 DoubleRow, 3 d-pair passes per 512-col chunk.
            # Batch 0 runs column-chunk-major so the left half starts as
            # soon as xbar chunk 0 lands; later batches run jp-major and
            # reuse loaded weights across the two column chunks.
            e = e_pool.tile([128, LT, L], bf16)
            for li in range(LT):
                ps = psum_s.tile([128, L], f32)
                lh = slice(128 * li, 128 * (li + 1))
                if WARM and bi == 0 and li == 0:
                    # Dummy matmuls during batch-0 staging: trip the PE HAM
                    # clock gate to 8/8 (~3.4us of activity) before the real
                    # S lands. They only depend on wz, so the PE runs them
                    # immediately; the real start=True group overwrites.
                    for wi in range(16):
                        nc.tensor.matmul(
                            ps[:, 0:512],
                            lhsT=wz[:, 0:128],
                            rhs=wz[:],
                            start=True,
                            stop=True,
                            skip_group_check=True,
                        )
                for outer in range(2 if bi == 0 else DP):
                    for inner in range(DP if bi == 0 else 2):
                        c, jp = (
                            (outer, inner) if bi == 0 else (inner, outer)
                        )
                        mm = nc.tensor.matmul(
                            ps[:, 512 * c : 512 * (c + 1)],
                            lhsT=at8[:, 2 * jp : 2 * jp + 2, lh],
                            rhs=at8[:, 2 * jp : 2 * jp + 2, 512 * c : 512 * (c + 1)],
                            start=(jp == 0),
                            stop=(jp == DP - 1),
                            perf_mode=DR,
                        )
                        if bi != 0 and c == 1:
                            mm.ins.ldweights = False
                sp = sp_pool.tile([128, L], f32)
                nc.vector.scalar_tensor_tensor(
                    out=sp[:],
                    in0=msk[:, li, :],
                    scalar=big_over_temp,
                    in1=ps[:],
                    op0=mybir.AluOpType.mult,
                    op1=mybir.AluOpType.add,
                )
                nc.scalar.activation(
                    out=e[:, li, :],
                    in_=sp[:],
                    func=mybir.ActivationFunctionType.Exp,
                    bias=neg_big[:],
                    scale=temp,
                )

            # [feat | denom] = E^T @ [t2 | 1]; normalize; store.
            for mi in range(LT):
                po = psum_o.tile([128, H + 1], f32, tag="po")
                for li in range(LT):
                    w = e[:, li, 128 * mi : 128 * (mi + 1)]
                    nc.tensor.matmul(
                        po[:, 0:512],
                        lhsT=w,
                        rhs=t2[:, li, 0:512],
                        start=(li == 0),
                        stop=(li == LT - 1),
                    )
                    mm2nd = nc.tensor.matmul(
                        po[:, 512 : H + 1],
                        lhsT=w,
                        rhs=t2[:, li, 512 : H + 1],
                        start=(li == 0),
                        stop=(li == LT - 1),
                    )
                    mm2nd.ins.ldweights = False
                rc = rc_pool.tile([128, 1], f32)
                nc.vector.reciprocal(rc[:], po[:, H : H + 1])
                ot = out_pool.tile([128, H], f32)
                if NORM_DVE and mi % 2 == 1:
                    nc.vector.tensor_scalar_mul(ot[:], po[:, 0:H], rc[:])
                else:
                    nc.scalar.activation(
                        out=ot[:],
                        in_=po[:, 0:H],
                        func=mybir.ActivationFunctionType.Copy,
                        scale=rc[:],
                    )
                if last:
                    out_eng = nc.scalar if mi % 2 == 0 else nc.sync
                else:
                    out_eng = nc.gpsimd
                out_eng.dma_start(out=o_v[:, mi, :], in_=ot[:])

        # Software pipeline: stage b+1 ahead of compute b.
        batches = [b for _ in range(repeats) for b in range(B_LOCAL)]
        staged = {0: emit_staging(0, batches[0])}
        for bi, b in enumerate(batches):
            if bi + 1 < len(batches):
                with tc.tile_wait_until(ST_DELAY, enable=bi == 0 and ST_DELAY > 0):
                    staged[bi + 1] = emit_staging(bi + 1, batches[bi + 1])
            emit_compute(bi, b, staged.pop(bi), last=bi == len(batches) - 1)

    nc.compile()
    return nc


def _get_nc(temp: float, repeats: int = 1, bench: bool = False):
    key = (round(float(temp), 12), repeats, bench)
    if key not in _CACHE:
        _CACHE[key] = _build(float(temp), repeats, bench)
    return _CACHE[key]


def run(a, mask_a, temperature=None, trace=False):
    from concourse.bass_utils import run_bass_kernel_spmd

    a = np.ascontiguousarray(np.asarray(a, dtype=np.float32))
    mask_u8 = np.ascontiguousarray(np.asarray(mask_a)).view(np.uint8)
    if temperature is None:
        temperature = 1.0 / np.sqrt(np.float32(H))
    temp = float(np.asarray(temperature, dtype=np.float32))

    nc = _get_nc(temp)
    in_maps = [
        {
            "a": a[c * B_LOCAL : (c + 1) * B_LOCAL],
            "mask_a": mask_u8[c * B_LOCAL : (c + 1) * B_LOCAL],
        }
        for c in range(N_CORES)
    ]
    res = run_bass_kernel_spmd(
        nc, in_maps, core_ids=list(range(N_CORES)), trace=trace
    )
    out = np.concatenate([res.results[c]["out"] for c in range(N_CORES)], axis=0)
    return out, res


def kernel(a, mask_a, temperature=None, **_):
    out, _res = run(a, mask_a, temperature)
    return out


# revision 33
# speedup vs baseline: 1.0026x; 1.0026x over previous
"""Masked self-attention (softmax over axis=1) Bass kernel for TRN2, 8 cores.

Reference semantics (per batch b):
    attn[l, m] = <a_l, a_m> * temperature            [L, L]
    attn = where(mask[l, m], attn, -1e7)
    P = softmax(attn, axis=l)                        (softmax over dim 0)
    out[m, :] = sum_l P[l, m] * a[l, :]              [L, H]

v4 design (pure data parallel, 4 batches/core, no collectives):

  Software-pipelined emission: staging for batch b+1 is emitted before
  compute of batch b, so the scheduler and the DMA ring FIFOs pull the
  staging chain a full batch ahead of where it's consumed.

  Staging per batch (chunked):
    asb f32 <- scalar-ring loads (batch 0: alternating sync/scalar)
    t2 bf16 [a|1] <- DVE/ACT casts per chunk
    scratch DRAM bf16 <- bounce t2 out on gpsimd SWDGE (batch 0:
        alternating scalar/gpsimd)
    AT bf16 <- xbar transpose DMAs (sync ring: transposes only after
        batch-0's early loads - mode switches serialize a ring)
    AT8 fp8e4 <- DVE casts per xbar chunk
    mask <- gpsimd SWDGE
  S = AT8^T @ AT8 in fp8 DoubleRow (2 d-tiles/pass, 3 passes)
  S' = mask_u8 * (BIG/temp) + S    (DVE scalar_tensor_tensor)
  E = exp(temp*S' - BIG) -> bf16   (ACT)
  [feat | denom] = E^T @ [t2 | 1]  (PE bf16, paired-chunk weight reuse)
  out = feat * (1/denom): DVE reciprocal + scale-copy alternating
      ACT / DVE per m-tile; stores alternate gpsimd/scalar rings.
  PE warmup: dummy matmuls during batch-0 staging flip the HAM clock
      gate to 8/8 before real work lands.
"""

import os as _os
import sys

import numpy as np

sys.path.insert(0, "/opt/trn_rl_repo")

B, L, H = 32, 1024, 768
N_CORES = 8
B_LOCAL = B // N_CORES  # 4 batches per core
LT = L // 128  # 8 l-tiles
DT = H // 128  # 6 d-tiles
DP = DT // 2  # 3 d-tile pairs (DoubleRow)
BIG = 50.0

N_TRANSPOSE = int(_os.environ.get("K_NT", "2"))  # xbar chunks per batch
N_STAGE = int(_os.environ.get("K_NST", "4"))  # a-load chunks
WARM = int(_os.environ.get("K_WARM", "1"))  # PE warmup matmuls
NORM_DVE = int(_os.environ.get("K_NDVE", "0"))  # keep normalize on ACT; DVE carries the transpose copies
ST_DELAY = float(_os.environ.get("K_STD", "0.010"))  # ms, staging-1 hold
MASK_DELAY = float(_os.environ.get("K_MKD", "0.014"))  # ms, mask-0 hold

_CACHE = {}


def _build(temp: float, repeats: int = 1, bench: bool = False):
    from contextlib import ExitStack

    import concourse.mybir as mybir
    from concourse import bacc, tile

    f32 = mybir.dt.float32
    bf16 = mybir.dt.bfloat16
    fp8 = mybir.dt.float8e4
    u8 = mybir.dt.uint8
    DR = mybir.MatmulPerfMode.DoubleRow

    nc = bacc.Bacc(
        "TRN2", target_bir_lowering=False, debug=False, num_devices=N_CORES
    )

    if bench:
        # Timing-only variant: big tensors live in Internal DRAM (content
        # irrelevant - instruction stream is identical), so per-call axon
        # transfer overhead stays tiny and the R-repeat delta is clean.
        nc.dram_tensor("bench_in", [1, 4], f32, kind="ExternalInput")
        nc.dram_tensor("out", [1, 4], f32, kind="ExternalOutput")
        a_ext = nc.dram_tensor("a", [B_LOCAL, L, H], f32).ap()
        m_ext = nc.dram_tensor("mask_a", [B_LOCAL, L, L], u8).ap()
        out_ext = nc.dram_tensor("out_int", [B_LOCAL, L, H], f32).ap()
    else:
        a_ext = nc.dram_tensor("a", [B_LOCAL, L, H], f32, kind="ExternalInput").ap()
        m_ext = nc.dram_tensor(
            "mask_a", [B_LOCAL, L, L], u8, kind="ExternalInput"
        ).ap()
        out_ext = nc.dram_tensor(
            "out", [B_LOCAL, L, H], f32, kind="ExternalOutput"
        ).ap()

    big_over_temp = BIG / temp

    with tile.TileContext(nc) as tc, ExitStack() as ctx:
        a_pool = ctx.enter_context(tc.tile_pool(name="asb", bufs=2))
        t2_pool = ctx.enter_context(tc.tile_pool(name="t2", bufs=3))
        at_pool = ctx.enter_context(tc.tile_pool(name="at", bufs=2))
        at8_pool = ctx.enter_context(tc.tile_pool(name="at8", bufs=2))
        mask_pool = ctx.enter_context(tc.tile_pool(name="mask", bufs=2))
        e_pool = ctx.enter_context(tc.tile_pool(name="e", bufs=2))
        sp_pool = ctx.enter_context(tc.tile_pool(name="sp", bufs=4))
        out_pool = ctx.enter_context(tc.tile_pool(name="outp", bufs=3))
        rc_pool = ctx.enter_context(tc.tile_pool(name="rc", bufs=3))
        dram_pool = ctx.enter_context(
            tc.tile_pool(name="bounce", bufs=2, space="DRAM")
        )
        psum_s = ctx.enter_context(tc.tile_pool(name="ps_s", bufs=2, space="PSUM"))
        psum_o = ctx.enter_context(tc.tile_pool(name="ps_o", bufs=2, space="PSUM"))
        const_pool = ctx.enter_context(tc.tile_pool(name="const", bufs=1))

        neg_big = const_pool.tile([128, 1], f32)
        nc.vector.memset(neg_big[:], -BIG)
        if WARM:
            wz = const_pool.tile([128, 512], bf16)
            nc.vector.memset(wz[:], 0.0)
        from concourse.masks import make_identity

        ident = const_pool.tile([128, 128], bf16)
        make_identity(nc, ident[:])

        def emit_staging(bi, b):
            a_v = a_ext[b].rearrange("(i p) d -> p i d", p=128)  # [128, 8, 768]
            m_v = m_ext[b].rearrange("(i p) m -> p i m", p=128)

            asb = a_pool.tile([128, LT, H], f32)
            t2 = t2_pool.tile([128, LT, H + 1], bf16)
            at8 = at8_pool.tile([128, DT, L], fp8)
            msk = mask_pool.tile([128, LT, L], u8)

            # a loads + t2 casts; batch 0 splits loads across both HWDGE
            # rings to halve the fill.
            lchunk = LT // N_STAGE
            for ci in range(N_STAGE):
                sl = slice(lchunk * ci, lchunk * (ci + 1))
                ld_eng = nc.sync if (bi == 0 and ci % 2 == 0) else nc.scalar
                ld_eng.dma_start(out=asb[:, sl, :], in_=a_v[:, sl, :])
                if ci % 2 == 0:
                    nc.vector.tensor_copy(t2[:, sl, 0:H], asb[:, sl, :])
                else:
                    nc.scalar.activation(
                        out=t2[:, sl, 0:H],
                        in_=asb[:, sl, :],
                        func=mybir.ActivationFunctionType.Copy,
                    )
            nc.vector.memset(t2[:, :, H : H + 1], 1.0)
            # PE block-transposes of t2 build AT8 for every batch: no
            # DRAM bounce, no xbar, no aliased DMA-semaphore waits. ~4.3us
            # of PE per batch, emitted ahead of the batch's S so the tile
            # scheduler streams them behind the previous batch's MM2.
            for dj in range(DT):
                tpf = psum_o.tile([128, H + 1], f32, tag="po")
                tp = tpf[:].bitcast(bf16)
                for li in range(LT):
                    nc.tensor.transpose(
                        tp[:, 128 * li : 128 * (li + 1)],
                        t2[:, li, 128 * dj : 128 * (dj + 1)],
                        ident[:],
                    )
                # one contiguous [128, 1024] PSUM->SBUF copy per d-tile
                nc.vector.tensor_copy(at8[:, dj, :], tp[:, 0:L])
            # mask (gpsimd SWDGE)
            with tc.tile_wait_until(MASK_DELAY, enable=bi == 0 and MASK_DELAY > 0):
                for ci in range(2):
                    sl = slice(4 * ci, 4 * (ci + 1))
                    nc.gpsimd.dma_start(out=msk[:, sl, :], in_=m_v[:, sl, :])
            return dict(t2=t2, at8=at8, msk=msk, bi=bi)

        def emit_compute(bi, b, st, last=False):
            t2, at8, msk = st["t2"], st["at8"], st["msk"]
            o_v = out_ext[b].rearrange("(i p) d -> p i d", p=128)
            # S rows: fp8 DoubleRow, 3 d-pair passes per 512-col chunk.
            # Batch 0 runs column-chunk-major so the left half starts as
            # soon as xbar chunk 0 lands; later batches run jp-major and
            # reuse loaded weights across the two column chunks.
            e = e_pool.tile([128, LT, L], bf16)
            for li in range(LT):
                ps = psum_s.tile([128, L], f32)
                lh = slice(128 * li, 128 * (li + 1))
                if WARM and bi == 0 and li == 0:
                    # Dummy matmuls during batch-0 staging: trip the PE HAM
                    # clock gate to 8/8 (~3.4us of activity) before the real
                    # S lands. They only depend on wz, so the PE runs them
                    # immediately; the real start=True group overwrites.
                    for wi in range(16):
                        nc.tensor.matmul(
                            ps[:, 0:512],
                            lhsT=wz[:, 0:128],
                            rhs=wz[:],
                            start=True,
                            stop=True,
                            skip_group_check=True,
                        )
                for outer in range(2 if bi == 0 else DP):
                    for inner in range(DP if bi == 0 else 2):
                        c, jp = (
                            (outer, inner) if bi == 0 else (inner, outer)
                        )
                        mm = nc.tensor.matmul(
                            ps[:, 512 * c : 512 * (c + 1)],
                            lhsT=at8[:, 2 * jp : 2 * jp + 2, lh],
                            rhs=at8[:, 2 * jp : 2 * jp + 2, 512 * c : 512 * (c + 1)],
                            start=(jp == 0),
                            stop=(jp == DP - 1),
                            perf_mode=DR,
                        )
                        if bi != 0 and c == 1:
                            mm.ins.ldweights = False
                sp = sp_pool.tile([128, L], f32)
                nc.vector.scalar_tensor_tensor(
                    out=sp[:],
                    in0=msk[:, li, :],
                    scalar=big_over_temp,
                    in1=ps[:],
                    op0=mybir.AluOpType.mult,
                    op1=mybir.AluOpType.add,
                )
                nc.scalar.activation(
                    out=e[:, li, :],
                    in_=sp[:],
                    func=mybir.ActivationFunctionType.Exp,
                    bias=neg_big[:],
                    scale=temp,
                )

            # [feat | denom] = E^T @ [t2 | 1]; normalize; store.
            for mi in range(LT):
                po = psum_o.tile([128, H + 1], f32, tag="po")
                for li in range(LT):
                    w = e[:, li, 128 * mi : 128 * (mi + 1)]
                    nc.tensor.matmul(
                        po[:, 0:512],
                        lhsT=w,
                        rhs=t2[:, li, 0:512],
                        start=(li == 0),
                        stop=(li == LT - 1),
                    )
                    mm2nd = nc.tensor.matmul(
                        po[:, 512 : H + 1],
                        lhsT=w,
                        rhs=t2[:, li, 512 : H + 1],
                        start=(li == 0),
                        stop=(li == LT - 1),
                    )
                    mm2nd.ins.ldweights = False
                rc = rc_pool.tile([128, 1], f32)
                nc.vector.reciprocal(rc[:], po[:, H : H + 1])
                ot = out_pool.tile([128, H], f32)
                if NORM_DVE and mi % 2 == 1:
                    nc.vector.tensor_scalar_mul(ot[:], po[:, 0:H], rc[:])
                else:
                    nc.scalar.activation(
                        out=ot[:],
                        in_=po[:, 0:H],
                        func=mybir.ActivationFunctionType.Copy,
                        scale=rc[:],
                    )
                if last:
                    out_eng = nc.scalar if mi % 2 == 0 else nc.sync
                else:
                    out_eng = nc.gpsimd
                out_eng.dma_start(out=o_v[:, mi, :], in_=ot[:])

        # Software pipeline: stage b+1 ahead of compute b.
        batches = [b for _ in range(repeats) for b in range(B_LOCAL)]
        staged = {0: emit_staging(0, batches[0])}
        for bi, b in enumerate(batches):
            if bi + 1 < len(batches):
                with tc.tile_wait_until(ST_DELAY, enable=bi == 0 and ST_DELAY > 0):
                    staged[bi + 1] = emit_staging(bi + 1, batches[bi + 1])
            emit_compute(bi, b, staged.pop(bi), last=bi == len(batches) - 1)

    nc.compile()
    return nc


def _get_nc(temp: float, repeats: int = 1, bench: bool = False):
    key = (round(float(temp), 12), repeats, bench)
    if key not in _CACHE:
        _CACHE[key] = _build(float(temp), repeats, bench)
    return _CACHE[key]


def run(a, mask_a, temperature=None, trace=False):
    from concourse.bass_utils import run_bass_kernel_spmd

    a = np.ascontiguousarray(np.asarray(a, dtype=np.float32))
    mask_u8 = np.ascontiguousarray(np.asarray(mask_a)).view(np.uint8)
    if temperature is None:
        temperature = 1.0 / np.sqrt(np.float32(H))
    temp = float(np.asarray(temperature, dtype=np.float32))

    nc = _get_nc(temp)
    in_maps = [
        {
            "a": a[c * B_LOCAL : (c + 1) * B_LOCAL],
            "mask_a": mask_u8[c * B_LOCAL : (c + 1) * B_LOCAL],
        }
        for c in range(N_CORES)
    ]
    res = run_bass_kernel_spmd(
        nc, in_maps, core_ids=list(range(N_CORES)), trace=trace
    )
    out = np.concatenate([res.results[c]["out"] for c in range(N_CORES)], axis=0)
    return out, res


def kernel(a, mask_a, temperature=None, **_):
    out, _res = run(a, mask_a, temperature)
    return out
